# revision 8
# baseline (speedup 1.0000x reference)
"""Trainium2 Bass kernel for nn_AdaptiveDirectionShift (v3).

Reference computation (B=16, C=320, H=W=64, G=5 groups of 64 channels):
  xn = zero-pad x spatially by 2          -> [B,C,68,68]
  em = mean_c(edge_guidance)              -> [B,1,64,64]
  h  = relu(conv3x3(em, w1, b1))          -> [B,16,64,64]
  dl = conv3x3(h, w2, b2)                 -> [B,2,64,64]
  dw = softmax(dl, axis=1)                -> wH = sigmoid(dl0-dl1), wW = 1-wH
  sh = roll rows of xn per group by shifts_h, crop -> [B,C,64,64]
  sw = roll cols of xn per group by shifts_w, crop
  out = wH*sh + wW*sw = sw + wH*(sh-sw)

Strategy: data-parallel over batch, 2 batches per core, no collectives.

v3 changes vs v2 (baseline 123.8us):
 * eg is ACCUMULATED over the 5 channel groups inside the casting DMAs
   (SWDGE CCE-add) into one [128, 4096] bf16 tile (b0 rows on partitions
   0-63, b1 on 64-127), so the channel-mean needs 8 ones-matmuls instead
   of 48.
 * the whole gate network runs batch-FUSED via block-diagonal weights:
   em [128->2], conv1 [18->32], conv2 [96->128]; sigmoid emits both
   batches' wh in one [128,512] activation per chunk.  PE work halves.
 * x tiles carry 2 full guard rows per side (GB=128 elems), so every
   row-shift |s|<=2 is a single flat offset read: no split runs, no row
   fixups.  Only tiny col-edge fixups (<=2 cols) remain.
 * groups with ODD col-shift get an ALIGNED copy of their shifted view
   (SBUF->SBUF DMA on the idle HWDGE ring): all combine TT ops then run
   in the DVE 2x bf16 mode instead of dropping to 1x on the 2-byte-
   misaligned operand (saves ~4us/group).
 * output stores and the aligned copies ride the sync HWDGE ring;
   taps/p2c patches ride scalar+sync; ONLY the casting loads use the
   gpsimd SWDGE ring.  The v2 store tail (23.5us Q7 drain) disappears.

Shift values are read host-side and baked into the access patterns
(compile-time specialization, like shapes).
"""

import numpy as np

B, C, H, W = 16, 320, 64, 64
HW = H * W
NCORES = 8
BLOC = B // NCORES  # 2 batches per core
G, CG = 5, 64       # channel groups
PAD = 2

PL = 66             # padded line width for conv tensors
FLAT = 64 * PL      # 4224: flat length of 64 rows of 66-wide lines
GB = 2 * W          # 128: two full guard rows each side of the x tiles
GTOT = GB + HW + GB  # 4352
P2L = PL * PL - 2   # 4354

# combine halves (A/B/C/store ranges): half0 must only need x rows 0-31
HALF0 = 30 * W      # 1920
HALVES = [(0, HALF0), (HALF0, HW)]
XH = HW // 2        # x loads come in 2048-elem halves

LAST_RESULT = {}


def _build(shifts_h, shifts_w):
    from contextlib import ExitStack

    import concourse.bass as bass
    import concourse.tile as tile
    from concourse import bacc, mybir

    f32 = mybir.dt.float32
    bf16 = mybir.dt.bfloat16
    nc = bacc.Bacc(None, target_bir_lowering=False)

    x_ext = nc.declare_dram_parameter("x", [BLOC, C, HW], f32, isOutput=False)
    eg_ext = nc.declare_dram_parameter("eg", [BLOC, C, HW], f32, isOutput=False)
    # block weights for the batch-fused gate (see kernel() for layouts)
    w1t2_ext = nc.declare_dram_parameter("w1t2", [18, 32], bf16, isOutput=False)
    dw2p2_ext = nc.declare_dram_parameter("dw2p2", [3 * 96, 128], bf16, isOutput=False)
    ones2_ext = nc.declare_dram_parameter("ones2", [128, 2], bf16, isOutput=False)
    b1x2_ext = nc.declare_dram_parameter("b1x2", [32, 1], f32, isOutput=False)
    db2_ext = nc.declare_dram_parameter("db2", [128, 1], f32, isOutput=False)
    out_ext = nc.declare_dram_parameter("out", [BLOC, C, HW], bf16, isOutput=True)

    sh_l = [int(v) for v in shifts_h]
    sw_l = [int(v) for v in shifts_w]
    assert all(-PAD <= v <= PAD for v in sh_l + sw_l)

    def raw_ap(tile_ap, part0, nparts, offset, free_dims):
        pstep = tile_ap.ap[0][0]
        return bass.AP(
            tensor=tile_ap.tensor,
            offset=tile_ap.offset + pstep * part0 + offset,
            ap=[[pstep, nparts]] + [list(d) for d in free_dims],
        )

    with tile.TileContext(nc) as tc, ExitStack() as ctx:
        singles = ctx.enter_context(tc.tile_pool(name="singles", bufs=1))
        gate_pool = ctx.enter_context(tc.tile_pool(name="gatep", bufs=1))
        # [128, HW] bf16 buffers: 5 eg group tiles + egsum, then the 5 e
        # tiles reuse the eg buffers (their lifetimes don't overlap)
        big = ctx.enter_context(tc.tile_pool(name="big", bufs=6))
        ps_em = ctx.enter_context(tc.tile_pool(name="psem", bufs=2, space="PSUM"))
        ps_h = ctx.enter_context(tc.tile_pool(name="psh", bufs=2, space="PSUM"))
        ps_d = ctx.enter_context(tc.tile_pool(name="psd", bufs=3, space="PSUM"))

        # ---- constants (tiny, on the sync HWDGE ring) ----
        ones2 = singles.tile([128, 2], bf16, tag="ones2")
        nc.sync.dma_start(out=ones2, in_=ones2_ext[:, :])
        w1t2 = singles.tile([18, 32], bf16, tag="w1t2")
        nc.sync.dma_start(out=w1t2, in_=w1t2_ext[:, :])
        dw2p2 = []
        for d in range(3):
            dwt = singles.tile([96, 128], bf16, tag=f"dw2p2{d}", name=f"dw2p2{d}")
            nc.sync.dma_start(out=dwt, in_=dw2p2_ext[96 * d : 96 * d + 96, :])
            dw2p2.append(dwt)
        b1x2 = singles.tile([32, 1], f32, tag="b1x2")
        nc.sync.dma_start(out=b1x2, in_=b1x2_ext[:, :])
        db2 = singles.tile([128, 1], f32, tag="db2")
        nc.sync.dma_start(out=db2, in_=db2_ext[:, :])

        # ---- persistent gate tensors ----
        # p1w2: partition 0/1 = em image of b0/b1 (64 rows x 66-wide lines),
        # partitions 2-9 = b0's other 8 conv taps, 10-17 = b1's.
        p1w2 = gate_pool.tile([18, FLAT], bf16, tag="p1w2")
        h_pad2 = gate_pool.tile([32, PL, PL], bf16, tag="h_pad2")
        blk = gate_pool.tile([96, P2L], bf16, tag="p2c")
        whb2 = gate_pool.tile([128, HW], bf16, tag="whb2")

        # zero only the strips the writers below do not cover
        nc.vector.memset(raw_ap(p1w2, 0, 18, 0, [[1, 67]]), 0.0)
        nc.vector.memset(raw_ap(p1w2, 0, 18, FLAT - 67, [[1, 67]]), 0.0)
        nc.vector.memset(raw_ap(p1w2, 0, 2, 0, [[PL, 64], [1, 1]]), 0.0)
        nc.vector.memset(raw_ap(p1w2, 0, 2, PL - 1, [[PL, 64], [1, 1]]), 0.0)
        nc.vector.memset(raw_ap(h_pad2, 0, 32, 0, [[1, PL]]), 0.0)
        nc.vector.memset(raw_ap(h_pad2, 0, 32, 65 * PL, [[1, PL]]), 0.0)
        nc.vector.memset(raw_ap(h_pad2, 0, 32, PL, [[PL, 64], [1, 1]]), 0.0)
        nc.vector.memset(raw_ap(h_pad2, 0, 32, PL + 65, [[PL, 64], [1, 1]]), 0.0)

        # dummy activation to pull the sigmoid ACT_TABLE load off the
        # critical path
        warm = gate_pool.tile([1, 1], f32, tag="warm")
        nc.vector.memset(warm, 0.0)
        nc.scalar.activation(
            out=warm, in_=warm, func=mybir.ActivationFunctionType.Sigmoid
        )

        # ---- x tiles + guard zeroing (DVE, cheap) ----
        xgs = []
        for g in range(G):
            xg = singles.tile([128, GTOT], bf16, tag=f"xg{g}", name=f"xg{g}")
            nc.vector.memset(raw_ap(xg, 0, 128, 0, [[1, GB]]), 0.0)
            nc.vector.memset(raw_ap(xg, 0, 128, GB + HW, [[1, GB]]), 0.0)
            xgs.append(xg)

        # ---- SWDGE ring (casting loads only), FIFO order = emission ----
        # eg: one [128, HW] tile per channel group (b0 on partitions 0-63,
        # b1 on 64-127); the group sum runs on the then-idle DVE as the
        # tiles land, so em is ready right after the last eg DMA.
        egts = []
        for g in range(G):
            egt = big.tile([128, HW], bf16, tag="bigbuf", name=f"egt{g}")
            for b in range(BLOC):
                nc.gpsimd.dma_start(
                    out=raw_ap(egt, b * CG, CG, 0, [[1, HW]]),
                    in_=eg_ext[b, g * CG : (g + 1) * CG, :],
                )
            egts.append(egt)
        # x: per group, per half (rows 0-31 / 32-63), both batches
        for g in range(G):
            for h in range(2):
                n0 = h * XH
                for b in range(BLOC):
                    nc.gpsimd.dma_start(
                        out=raw_ap(xgs[g], b * CG, CG, GB + n0, [[1, XH]]),
                        in_=x_ext[b, g * CG : (g + 1) * CG, n0 : n0 + XH],
                    )

        # ================= gate network (batch-fused) ==================
        # group-sum chain on the DVE (idle until the combines start)
        egsum = big.tile([128, HW], bf16, tag="bigbuf", name="egsum")
        nc.vector.tensor_add(egsum, egts[0], egts[1])
        for g in range(2, G):
            nc.vector.tensor_add(egsum, egsum, egts[g])

        # em: channel sums of both batches in one pass; 1/C lives in w1t2
        for j in range(HW // 512):
            em_ps = ps_em.tile([2, 512], f32, tag="em_ps")
            nc.tensor.matmul(
                em_ps, ones2, egsum[:, j * 512 : (j + 1) * 512],
                start=True, stop=True,
            )
            r0 = (j * 512) // W
            dst = raw_ap(p1w2, 0, 2, r0 * PL + 1, [[PL, 8], [1, 64]])
            nc.scalar.copy(
                out=dst,
                in_=em_ps[0:2, :].rearrange("p (r c) -> p r c", c=64),
            )

        # taps: build the 8 shifted copies of each em image.  Per batch:
        # src = em partition (0 or 1) with a free-dim tap expansion, dst =
        # consecutive tap partitions.  Split across both HWDGE rings.
        HCUT = 32 * PL
        TAPGROUPS = (
            (nc.sync, 0, 3, -PL - 1, 1),   # taps 0,1,2: delta -67,-66,-65
            (nc.scalar, 3, 2, -1, 2),      # taps 3,5:   delta -1,+1
            (nc.sync, 5, 3, PL - 1, 1),    # taps 6,7,8: delta 65,66,67
        )
        for half in range(2):
            for b in range(BLOC):
                for ring, tapoff, np_, base, estep in TAPGROUPS:
                    lo = max(0, -base)
                    hi = FLAT - max(0, base + estep * (np_ - 1))
                    a0, a1 = (lo, HCUT) if half == 0 else (HCUT, hi)
                    src = raw_ap(
                        p1w2, b, 1, a0 + base, [[estep, np_], [1, a1 - a0]]
                    )
                    dst = raw_ap(p1w2, 2 + 8 * b + tapoff, np_, a0, [[1, a1 - a0]])
                    ring.dma_start(out=dst, in_=src)

        # conv1: both batches per pass via block-diag w1t2 [18 -> 32]
        for j in range(HW // 512):
            h_ps = ps_h.tile([32, 512], f32, tag="h_ps")
            r0 = (j * 512) // W
            rhs = raw_ap(p1w2, 0, 18, r0 * PL + 1, [[PL, 8], [1, 64]])
            nc.tensor.matmul(h_ps, w1t2, rhs, start=True, stop=True)
            nc.scalar.activation(
                out=h_pad2[0:32, 1 + r0 : 9 + r0, 1:65],
                in_=h_ps[0:32, :].rearrange("p (r c) -> p r c", c=64),
                func=mybir.ActivationFunctionType.Relu,
                bias=b1x2[0:32, 0:1],
            )

        # p2c: full-image K=96 patch (b0 -> partitions 0-47, b1 -> 48-95),
        # halves split across the two HWDGE rings
        P2H = P2L // 2
        for b in range(BLOC):
            for half, ring in ((0, nc.sync), (1, nc.scalar)):
                a0 = half * P2H
                a1 = P2L if half else P2H
                ring.dma_start(
                    out=raw_ap(blk, 48 * b, 48, a0, [[1, a1 - a0]]),
                    in_=raw_ap(h_pad2, 16 * b, 16, a0, [[1, 3], [1, a1 - a0]]),
                )

        # conv2 + sigmoid: [96 -> 128] block-diag, 3 accumulating passes
        for j in range(HW // 512):
            r0 = (j * 512) // W
            d_ps = ps_d.tile([128, 512], f32, tag="d_ps")
            for d in range(3):
                rhs = raw_ap(blk, 0, 96, (r0 + d) * PL, [[PL, 8], [1, 64]])
                nc.tensor.matmul(
                    d_ps, dw2p2[d], rhs, start=(d == 0), stop=(d == 2)
                )
            nc.scalar.activation(
                out=raw_ap(whb2, 0, 128, j * 512, [[1, 512]]),
                in_=d_ps[0:128, :],
                func=mybir.ActivationFunctionType.Sigmoid,
                bias=db2[0:128, 0:1],
            )

        # ================= shifted combines (per group) ================
        # Per group g: sh = flat read at GB - W*s, sw = flat read at
        # GB - t (aligned copy swc for odd t).  All on the DVE:
        #   A: e = sh - sw       B: e *= wh       C: e += sw
        #   fixup: cols where sw==0: e = wh*sh
        eng = nc.vector
        # e tiles reuse the eg tile buffers (big pool, same tag)
        e_ts = [
            big.tile([128, HW], bf16, tag="bigbuf", name=f"e{g}")
            for g in range(G)
        ]
        swcs = {}
        for g in range(G):
            if sw_l[g] % 2 != 0:
                swcs[g] = singles.tile(
                    [128, HW], bf16, tag=f"swc{g}", name=f"swc{g}"
                )

        def swc_copy(g, h):
            lo, hi = HALVES[h]
            nc.sync.dma_start(
                out=raw_ap(swcs[g], 0, 128, lo, [[1, hi - lo]]),
                in_=raw_ap(xgs[g], 0, 128, GB - sw_l[g] + lo, [[1, hi - lo]]),
            )

        def sw_ap(g, lo, hi):
            if g in swcs:
                return raw_ap(swcs[g], 0, 128, lo, [[1, hi - lo]])
            return raw_ap(xgs[g], 0, 128, GB - sw_l[g] + lo, [[1, hi - lo]])

        def a_op(g, h):
            lo, hi = HALVES[h]
            eng.tensor_sub(
                raw_ap(e_ts[g], 0, 128, lo, [[1, hi - lo]]),
                raw_ap(xgs[g], 0, 128, GB - W * sh_l[g] + lo, [[1, hi - lo]]),
                sw_ap(g, lo, hi),
            )

        def b_op(g, h):
            lo, hi = HALVES[h]
            eng.tensor_mul(
                raw_ap(e_ts[g], 0, 128, lo, [[1, hi - lo]]),
                raw_ap(e_ts[g], 0, 128, lo, [[1, hi - lo]]),
                raw_ap(whb2, 0, 128, lo, [[1, hi - lo]]),
            )

        def c_op(g, h):
            lo, hi = HALVES[h]
            eng.tensor_add(
                raw_ap(e_ts[g], 0, 128, lo, [[1, hi - lo]]),
                raw_ap(e_ts[g], 0, 128, lo, [[1, hi - lo]]),
                sw_ap(g, lo, hi),
            )

        def fix_op(g, h):
            # cols where sw is conceptually zero: out = wh * sh (valid for
            # every row: at sh-guard rows both terms are zero)
            t = sw_l[g]
            if t == 0:
                return
            rlo, rhi = (0, 30) if h == 0 else (30, 64)
            nr = rhi - rlo
            j0, nj = (0, t) if t > 0 else (64 + t, -t)
            f0 = rlo * W + j0
            er = raw_ap(e_ts[g], 0, 128, f0, [[W, nr], [1, nj]])
            whr = raw_ap(whb2, 0, 128, f0, [[W, nr], [1, nj]])
            shr = raw_ap(
                xgs[g], 0, 128, GB - W * sh_l[g] + f0, [[W, nr], [1, nj]]
            )
            eng.tensor_mul(er, whr, shr)

        def store(g, h):
            lo, hi = HALVES[h]
            nc.sync.dma_start(
                out=out_ext[:, g * CG : (g + 1) * CG, lo:hi],
                in_=raw_ap(e_ts[g], 0, 128, lo, [[1, hi - lo]]),
            )

        # Readiness-ordered emission.  DVE is in-order: emit each group's
        # A as its x tile lands, B/C/fix behind the sigmoid, stores chase.
        for g in (0, 1):
            if g in swcs:
                swc_copy(g, 0)
                swc_copy(g, 1)
            a_op(g, 0)
            a_op(g, 1)

        def bcf(g):
            for h in range(2):
                b_op(g, h)
                c_op(g, h)
                fix_op(g, h)
                store(g, h)

        bcf(0)
        if 2 in swcs:
            swc_copy(2, 0)
            swc_copy(2, 1)
        a_op(2, 0)
        a_op(2, 1)
        bcf(1)
        if 3 in swcs:
            swc_copy(3, 0)
            swc_copy(3, 1)
        a_op(3, 0)
        a_op(3, 1)
        bcf(2)
        if 4 in swcs:
            swc_copy(4, 0)
            swc_copy(4, 1)
        a_op(4, 0)
        a_op(4, 1)
        bcf(3)
        bcf(4)

    nc.finalize()
    return nc


_GRAPH_CACHE = {}


def _install_ntff_hook_shim():
    """The agent image's ``antenv`` lacks ``axon_hooks``; recreate it so
    run_bass_kernel_spmd(trace=True) can capture NTFF profiles."""
    import sys
    import types

    if "antenv.axon_hooks" in sys.modules:
        return
    try:
        import antenv
        from trn_agent_boot.trn_boot import _ntff_profile_via_ctypes
    except ImportError:
        return
    hook = _ntff_profile_via_ctypes("/opt/axon/libaxon_pjrt.so")
    mod = types.ModuleType("antenv.axon_hooks")
    mod._hook = hook

    def set_axon_ntff_profile_hook(h):
        mod._hook = h

    def get_axon_ntff_profile_hook():
        return mod._hook

    mod.set_axon_ntff_profile_hook = set_axon_ntff_profile_hook
    mod.get_axon_ntff_profile_hook = get_axon_ntff_profile_hook
    sys.modules["antenv.axon_hooks"] = mod
    antenv.axon_hooks = mod


def kernel(**inputs):
    from concourse.bass_utils import run_bass_kernel_spmd

    _install_ntff_hook_shim()

    x = np.ascontiguousarray(inputs["x"], dtype=np.float32).reshape(B, C, HW)
    eg = np.ascontiguousarray(inputs["edge_guidance"], dtype=np.float32).reshape(
        B, C, HW
    )
    w1 = np.asarray(inputs["w1"], dtype=np.float32).reshape(16, 9)
    b1 = np.asarray(inputs["b1"], dtype=np.float32).reshape(16)
    w2 = np.asarray(inputs["w2"], dtype=np.float32).reshape(2, 16, 9)
    b2 = np.asarray(inputs["b2"], dtype=np.float32).reshape(2)
    shifts_h = np.asarray(inputs["shifts_h"]).astype(np.int64)
    shifts_w = np.asarray(inputs["shifts_w"]).astype(np.int64)

    import ml_dtypes

    # gate constants (batch-fused block-diagonal layouts)
    # w1t2 [18, 32]: rows 0/1 = center tap (em_b0/em_b1), rows 2-9/10-17 =
    # taps [0,1,2,3,5,6,7,8] of b0/b1; cols 0-15 = b0 outputs, 16-31 = b1.
    # The 1/C edge-mean scale is folded in (p1w2 holds channel sums).
    w1tc = (w1.T / C).astype(np.float32)  # [9, 16]
    perm8 = [0, 1, 2, 3, 5, 6, 7, 8]
    w1t2 = np.zeros((18, 32), np.float32)
    w1t2[0, 0:16] = w1tc[4]
    w1t2[1, 16:32] = w1tc[4]
    for k, tap in enumerate(perm8):
        w1t2[2 + k, 0:16] = w1tc[tap]
        w1t2[10 + k, 16:32] = w1tc[tap]
    w1t2 = w1t2.astype(ml_dtypes.bfloat16)

    # dw2p2 [3][96, 128]: row 3c+dj = (w2[0]-w2[1])[c, 3d+dj] replicated to
    # 64 cols (b0 block), rows 48+.. same for b1 in cols 64-127.
    dw2 = (w2[0] - w2[1]).reshape(16, 3, 3)
    dw2p2 = np.zeros((3, 96, 128), np.float32)
    for d in range(3):
        base = dw2[:, d, :].reshape(48, 1)  # row 3c+dj
        dw2p2[d, 0:48, 0:64] = base
        dw2p2[d, 48:96, 64:128] = base
    dw2p2 = dw2p2.reshape(288, 128).astype(ml_dtypes.bfloat16)

    ones2 = np.zeros((128, 2), np.float32)
    ones2[0:64, 0] = 1.0
    ones2[64:128, 1] = 1.0
    ones2 = ones2.astype(ml_dtypes.bfloat16)

    b1x2 = np.tile(b1, 2).reshape(32, 1).astype(np.float32)
    db2 = np.full((128, 1), b2[0] - b2[1], dtype=np.float32)

    key = (tuple(shifts_h.tolist()), tuple(shifts_w.tolist()))
    if key not in _GRAPH_CACHE:
        _GRAPH_CACHE[key] = _build(shifts_h, shifts_w)
    nc = _GRAPH_CACHE[key]

    in_maps = []
    for i in range(NCORES):
        bsl = slice(i * BLOC, (i + 1) * BLOC)
        in_maps.append(
            {
                "x": np.ascontiguousarray(x[bsl]),
                "eg": np.ascontiguousarray(eg[bsl]),
                "w1t2": w1t2,
                "dw2p2": dw2p2,
                "ones2": ones2,
                "b1x2": b1x2,
                "db2": db2,
            }
        )

    import os

    trace = os.environ.get("BASS_KERNEL_TRACE", "0") == "1"
    try:
        res = run_bass_kernel_spmd(nc, in_maps, list(range(NCORES)), trace=trace)
    except Exception:
        if not trace:
            raise
        res = run_bass_kernel_spmd(nc, in_maps, list(range(NCORES)), trace=False)

    LAST_RESULT["exec_time_ns"] = getattr(res, "exec_time_ns", None)
    LAST_RESULT["profile_json"] = getattr(res, "profile_json", None)

    out = np.concatenate(
        [np.asarray(res.results[i]["out"]) for i in range(NCORES)], axis=0
    )
    return out.astype(np.float32).reshape(B, C, H, W)


# revision 9
# speedup vs baseline: 1.3236x; 1.3236x over previous
"""Trainium2 Bass kernel for nn_AdaptiveDirectionShift (v4).

Reference computation (B=16, C=320, H=W=64, G=5 groups of 64 channels):
  xn = zero-pad x spatially by 2          -> [B,C,68,68]
  em = mean_c(edge_guidance)              -> [B,1,64,64]
  h  = relu(conv3x3(em, w1, b1))          -> [B,16,64,64]
  dl = conv3x3(h, w2, b2)                 -> [B,2,64,64]
  dw = softmax(dl, axis=1)                -> wH = sigmoid(dl0-dl1), wW = 1-wH
  sh = roll rows of xn per group by shifts_h, crop -> [B,C,64,64]
  sw = roll cols of xn per group by shifts_w, crop
  out = wH*sh + wW*sw = sw + wH*(sh-sw)

Strategy: data-parallel over batch, 2 batches per core, no collectives.

v4 design (baseline v2: 123.8us; v3: 195.8us — HWDGE DRAM stores turned
out to cost ~10us of ucode descriptor generation each):
 * ALL HBM traffic (casting loads + stores) on the gpsimd SWDGE ring,
   whose Q7 CounterMachine generates the 128-row DRAM descriptors fast.
   HWDGE rings carry only SBUF->SBUF traffic (taps, conv patches,
   aligned shifted copies) where their descriptor RTL is fast.
 * batch-fused gate via block-diagonal weights: em [128->2] with a
   [128,2] ones mask, conv1 [18->32], conv2 [96->128]; one sigmoid per
   512-chunk emits both batches' wh.
 * em accumulates over the 5 channel groups ON THE PE: 8 PSUM chunk
   tiles stay open while per-group matmuls accumulate as each eg tile
   lands.  Costs zero DVE time.
 * the gate is pipelined in image-row halves: eg loads arrive in
   halves, and em/taps/conv1/p2c/conv2/sigmoid chase so the first
   sigmoid chunks are ready ~25us before the last eg byte's full-image
   gate would be.
 * x tiles carry 2 full guard rows per side (GB=128), so every row
   shift |s|<=2 is one flat offset read; groups with ODD col-shift get
   an aligned SBUF->SBUF copy so all DVE combine ops run in 2x bf16
   mode.  Only tiny col-edge fixups remain.
 * the Pool engine takes over C (+fixups) for two late-arriving groups
   once its load-dispatch work is done, shortening the DVE tail; the
   last-loaded group is the one needing no aligned copy.

Shift values are read host-side and baked into the access patterns
(compile-time specialization, like shapes).
"""

import numpy as np

B, C, H, W = 16, 320, 64, 64
HW = H * W
NCORES = 8
BLOC = B // NCORES  # 2 batches per core
G, CG = 5, 64       # channel groups
PAD = 2

PL = 66             # padded line width for conv tensors
FLAT = 64 * PL      # 4224: flat length of 64 rows of 66-wide lines
GB = 2 * W          # 128: two full guard rows each side of the x tiles
GTOT = GB + HW + GB  # 4352
P2L = PL * PL - 2   # 4354

# combine halves (A/B/C/store ranges): half0 must only need x rows 0-31
HALF0 = 30 * W      # 1920
HALVES = [(0, HALF0), (HALF0, HW)]
XH = HW // 2        # loads come in 2048-elem (32-row) halves

# gate pipeline half boundaries (see derivation in comments below)
TAPS_CUT = 30 * PL   # taps half0 reads em rows <= 31
P2C_CUT = 25 * PL - 2  # p2c half0 reads h_pad2 rows <= 24 (relu chunks 0-2)
CONV1_H0 = 3         # conv1 chunks 0-2 need taps half0 only
CONV2_H0 = 2         # conv2 chunks 0-1 need p2c half0 only

LAST_RESULT = {}


def _build(shifts_h, shifts_w):
    from contextlib import ExitStack

    import concourse.bass as bass
    import concourse.tile as tile
    from concourse import bacc, mybir

    f32 = mybir.dt.float32
    bf16 = mybir.dt.bfloat16
    nc = bacc.Bacc(None, target_bir_lowering=False)

    x_ext = nc.declare_dram_parameter("x", [BLOC, C, HW], f32, isOutput=False)
    eg_ext = nc.declare_dram_parameter("eg", [BLOC, C, HW], f32, isOutput=False)
    w1t2_ext = nc.declare_dram_parameter("w1t2", [18, 32], bf16, isOutput=False)
    dw2p2_ext = nc.declare_dram_parameter("dw2p2", [3 * 96, 128], bf16, isOutput=False)
    ones2_ext = nc.declare_dram_parameter("ones2", [128, 2], bf16, isOutput=False)
    b1x2_ext = nc.declare_dram_parameter("b1x2", [32, 1], f32, isOutput=False)
    db2_ext = nc.declare_dram_parameter("db2", [128, 1], f32, isOutput=False)
    out_ext = nc.declare_dram_parameter("out", [BLOC, C, HW], bf16, isOutput=True)

    sh_l = [int(v) for v in shifts_h]
    sw_l = [int(v) for v in shifts_w]
    assert all(-PAD <= v <= PAD for v in sh_l + sw_l)

    # load/combine order: put a group needing no aligned copy (even t)
    # last if one exists — the last-arriving group has the shortest
    # post-arrival chain.  The 3rd/4th groups get their C on the Pool.
    evens = [g for g in range(G) if sw_l[g] % 2 == 0]
    odds = [g for g in range(G) if sw_l[g] % 2 != 0]
    gorder = odds + evens if evens else list(range(G))
    pool_c = set(gorder[2:4])

    def raw_ap(tile_ap, part0, nparts, offset, free_dims):
        pstep = tile_ap.ap[0][0]
        return bass.AP(
            tensor=tile_ap.tensor,
            offset=tile_ap.offset + pstep * part0 + offset,
            ap=[[pstep, nparts]] + [list(d) for d in free_dims],
        )

    with tile.TileContext(nc) as tc, ExitStack() as ctx:
        singles = ctx.enter_context(tc.tile_pool(name="singles", bufs=1))
        gate_pool = ctx.enter_context(tc.tile_pool(name="gatep", bufs=1))
        # [128, HW] bf16 buffers: 5 eg group tiles, then the 5 e tiles
        # reuse them (lifetimes don't overlap)
        big = ctx.enter_context(tc.tile_pool(name="big", bufs=5))
        # single PSUM pool: 8 em chunk tiles live at once, then conv1/
        # conv2 tiles cycle through the freed banks
        ps = ctx.enter_context(tc.tile_pool(name="ps", bufs=8, space="PSUM"))

        # ---- constants (tiny, on the sync HWDGE ring) ----
        ones2 = singles.tile([128, 2], bf16, tag="ones2")
        nc.sync.dma_start(out=ones2, in_=ones2_ext[:, :])
        w1t2 = singles.tile([18, 32], bf16, tag="w1t2")
        nc.sync.dma_start(out=w1t2, in_=w1t2_ext[:, :])
        dw2p2 = []
        for d in range(3):
            dwt = singles.tile([96, 128], bf16, tag=f"dw2p2{d}", name=f"dw2p2{d}")
            nc.sync.dma_start(out=dwt, in_=dw2p2_ext[96 * d : 96 * d + 96, :])
            dw2p2.append(dwt)
        b1x2 = singles.tile([32, 1], f32, tag="b1x2")
        nc.sync.dma_start(out=b1x2, in_=b1x2_ext[:, :])
        db2 = singles.tile([128, 1], f32, tag="db2")
        nc.sync.dma_start(out=db2, in_=db2_ext[:, :])

        # ---- persistent gate tensors ----
        # p1w2: partitions 0/1 = em images of b0/b1 (64 rows x 66-wide
        # lines), partitions 2-9 = b0's 8 other conv taps, 10-17 = b1's.
        p1w2 = gate_pool.tile([18, FLAT], bf16, tag="p1w2")
        h_pad2 = gate_pool.tile([32, PL, PL], bf16, tag="h_pad2")
        blk = gate_pool.tile([96, P2L], bf16, tag="p2c")
        whb2 = gate_pool.tile([128, HW], bf16, tag="whb2")

        # zero only the strips the writers below do not cover
        nc.vector.memset(raw_ap(p1w2, 0, 18, 0, [[1, 67]]), 0.0)
        nc.vector.memset(raw_ap(p1w2, 0, 18, FLAT - 67, [[1, 67]]), 0.0)
        nc.vector.memset(raw_ap(p1w2, 0, 2, 0, [[PL, 64], [1, 1]]), 0.0)
        nc.vector.memset(raw_ap(p1w2, 0, 2, PL - 1, [[PL, 64], [1, 1]]), 0.0)
        nc.vector.memset(raw_ap(h_pad2, 0, 32, 0, [[1, PL]]), 0.0)
        nc.vector.memset(raw_ap(h_pad2, 0, 32, 65 * PL, [[1, PL]]), 0.0)
        nc.vector.memset(raw_ap(h_pad2, 0, 32, PL, [[PL, 64], [1, 1]]), 0.0)
        nc.vector.memset(raw_ap(h_pad2, 0, 32, PL + 65, [[PL, 64], [1, 1]]), 0.0)

        # dummy activation pulls the sigmoid ACT_TABLE load off the path
        warm = gate_pool.tile([1, 1], f32, tag="warm")
        nc.vector.memset(warm, 0.0)
        nc.scalar.activation(
            out=warm, in_=warm, func=mybir.ActivationFunctionType.Sigmoid
        )

        # ---- x tiles + guard zeroing (DVE, cheap) ----
        xgs = {}
        for g in range(G):
            xg = singles.tile([128, GTOT], bf16, tag=f"xg{g}", name=f"xg{g}")
            nc.vector.memset(raw_ap(xg, 0, 128, 0, [[1, GB]]), 0.0)
            nc.vector.memset(raw_ap(xg, 0, 128, GB + HW, [[1, GB]]), 0.0)
            xgs[g] = xg

        # ---- SWDGE ring: eg halves, then x (gorder), stores come later
        egts = []
        for g in range(G):
            egts.append(big.tile([128, HW], bf16, tag="bigbuf", name=f"egt{g}"))
        for h in range(2):
            n0 = h * XH
            for g in range(G):
                for b in range(BLOC):
                    nc.gpsimd.dma_start(
                        out=raw_ap(egts[g], b * CG, CG, n0, [[1, XH]]),
                        in_=eg_ext[b, g * CG : (g + 1) * CG, n0 : n0 + XH],
                    )
        for g in gorder:
            for h in range(2):
                n0 = h * XH
                for b in range(BLOC):
                    nc.gpsimd.dma_start(
                        out=raw_ap(xgs[g], b * CG, CG, GB + n0, [[1, XH]]),
                        in_=x_ext[b, g * CG : (g + 1) * CG, n0 : n0 + XH],
                    )

        # ================= gate network (batch-fused, half-pipelined) ==
        NJ = HW // 512  # 8 chunks of 512 px (8 image rows each)
        em_ps = [None] * NJ

        def em_stage(jlo, jhi):
            # accumulate the channel sums on the PE: per group, chunk
            # matmuls [128 -> 2] land in open PSUM tiles as eg arrives
            for j in range(jlo, jhi):
                em_ps[j] = ps.tile([2, 512], f32, tag="ps", name=f"em{j}")
            for g in range(G):
                for j in range(jlo, jhi):
                    nc.tensor.matmul(
                        em_ps[j], ones2, egts[g][:, j * 512 : (j + 1) * 512],
                        start=(g == 0), stop=(g == G - 1),
                    )
            for j in range(jlo, jhi):
                r0 = (j * 512) // W
                dst = raw_ap(p1w2, 0, 2, r0 * PL + 1, [[PL, 8], [1, 64]])
                nc.scalar.copy(
                    out=dst,
                    in_=em_ps[j][0:2, :].rearrange("p (r c) -> p r c", c=64),
                )

        TAPGROUPS = (
            (nc.sync, 0, 3, -PL - 1, 1),   # taps 0,1,2: delta -67,-66,-65
            (nc.scalar, 3, 2, -1, 2),      # taps 3,5:   delta -1,+1
            (nc.sync, 5, 3, PL - 1, 1),    # taps 6,7,8: delta 65,66,67
        )

        def taps_stage(half):
            for b in range(BLOC):
                for ring, tapoff, np_, base, estep in TAPGROUPS:
                    lo = max(0, -base)
                    hi = FLAT - max(0, base + estep * (np_ - 1))
                    a0, a1 = (lo, TAPS_CUT) if half == 0 else (TAPS_CUT, hi)
                    src = raw_ap(
                        p1w2, b, 1, a0 + base, [[estep, np_], [1, a1 - a0]]
                    )
                    dst = raw_ap(p1w2, 2 + 8 * b + tapoff, np_, a0, [[1, a1 - a0]])
                    ring.dma_start(out=dst, in_=src)

        def conv1_stage(jlo, jhi):
            for j in range(jlo, jhi):
                h_ps = ps.tile([32, 512], f32, tag="ps", name=f"h{j}")
                r0 = (j * 512) // W
                rhs = raw_ap(p1w2, 0, 18, r0 * PL + 1, [[PL, 8], [1, 64]])
                nc.tensor.matmul(h_ps, w1t2, rhs, start=True, stop=True)
                nc.scalar.activation(
                    out=h_pad2[0:32, 1 + r0 : 9 + r0, 1:65],
                    in_=h_ps[0:32, :].rearrange("p (r c) -> p r c", c=64),
                    func=mybir.ActivationFunctionType.Relu,
                    bias=b1x2[0:32, 0:1],
                )

        def p2c_stage(half):
            for b, ring in ((0, nc.sync), (1, nc.scalar)):
                a0, a1 = (0, P2C_CUT) if half == 0 else (P2C_CUT, P2L)
                ring.dma_start(
                    out=raw_ap(blk, 48 * b, 48, a0, [[1, a1 - a0]]),
                    in_=raw_ap(h_pad2, 16 * b, 16, a0, [[1, 3], [1, a1 - a0]]),
                )

        def conv2_stage(jlo, jhi):
            for j in range(jlo, jhi):
                r0 = (j * 512) // W
                d_ps = ps.tile([128, 512], f32, tag="ps", name=f"d{j}")
                for d in range(3):
                    rhs = raw_ap(blk, 0, 96, (r0 + d) * PL, [[PL, 8], [1, 64]])
                    nc.tensor.matmul(
                        d_ps, dw2p2[d], rhs, start=(d == 0), stop=(d == 2)
                    )
                nc.scalar.activation(
                    out=raw_ap(whb2, 0, 128, j * 512, [[1, 512]]),
                    in_=d_ps[0:128, :],
                    func=mybir.ActivationFunctionType.Sigmoid,
                    bias=db2[0:128, 0:1],
                )

        # half-pipelined gate: the h0 stages run while eg h1 streams in
        em_stage(0, NJ // 2)
        taps_stage(0)
        conv1_stage(0, CONV1_H0)
        p2c_stage(0)
        conv2_stage(0, CONV2_H0)
        em_stage(NJ // 2, NJ)
        taps_stage(1)
        conv1_stage(CONV1_H0, NJ)
        p2c_stage(1)
        conv2_stage(CONV2_H0, NJ)

        # ================= shifted combines (per group) ================
        # Per group g: sh = flat read at GB - W*s, sw = flat read at
        # GB - t (aligned copy swc for odd t).
        #   A: e = sh - sw       B: e *= wh       C: e += sw
        #   fixup: cols where sw==0: e = wh*sh
        # A/B on the DVE; C+fixup on the Pool for pool_c groups.
        e_ts = {}
        for g in gorder:
            e_ts[g] = big.tile([128, HW], bf16, tag="bigbuf", name=f"e{g}")
        swcs = {}
        for g in odds:
            swcs[g] = singles.tile([128, HW], bf16, tag=f"swc{g}", name=f"swc{g}")

        def swc_copy(g, h):
            lo, hi = HALVES[h]
            nc.sync.dma_start(
                out=raw_ap(swcs[g], 0, 128, lo, [[1, hi - lo]]),
                in_=raw_ap(xgs[g], 0, 128, GB - sw_l[g] + lo, [[1, hi - lo]]),
            )

        def sw_ap(g, lo, hi):
            if g in swcs:
                return raw_ap(swcs[g], 0, 128, lo, [[1, hi - lo]])
            return raw_ap(xgs[g], 0, 128, GB - sw_l[g] + lo, [[1, hi - lo]])

        def a_op(g, h):
            lo, hi = HALVES[h]
            nc.vector.tensor_sub(
                raw_ap(e_ts[g], 0, 128, lo, [[1, hi - lo]]),
                raw_ap(xgs[g], 0, 128, GB - W * sh_l[g] + lo, [[1, hi - lo]]),
                sw_ap(g, lo, hi),
            )

        def b_op(g, h):
            lo, hi = HALVES[h]
            nc.vector.tensor_mul(
                raw_ap(e_ts[g], 0, 128, lo, [[1, hi - lo]]),
                raw_ap(e_ts[g], 0, 128, lo, [[1, hi - lo]]),
                raw_ap(whb2, 0, 128, lo, [[1, hi - lo]]),
            )

        def c_op(eng, g, h):
            lo, hi = HALVES[h]
            eng.tensor_add(
                raw_ap(e_ts[g], 0, 128, lo, [[1, hi - lo]]),
                raw_ap(e_ts[g], 0, 128, lo, [[1, hi - lo]]),
                sw_ap(g, lo, hi),
            )

        def fix_op(eng, g, h):
            t = sw_l[g]
            if t == 0:
                return
            rlo, rhi = (0, 30) if h == 0 else (30, 64)
            nr = rhi - rlo
            j0, nj = (0, t) if t > 0 else (64 + t, -t)
            f0 = rlo * W + j0
            er = raw_ap(e_ts[g], 0, 128, f0, [[W, nr], [1, nj]])
            whr = raw_ap(whb2, 0, 128, f0, [[W, nr], [1, nj]])
            shr = raw_ap(
                xgs[g], 0, 128, GB - W * sh_l[g] + f0, [[W, nr], [1, nj]]
            )
            eng.tensor_mul(er, whr, shr)

        def store(g, h):
            lo, hi = HALVES[h]
            nc.gpsimd.dma_start(
                out=out_ext[:, g * CG : (g + 1) * CG, lo:hi],
                in_=raw_ap(e_ts[g], 0, 128, lo, [[1, hi - lo]]),
            )

        # Readiness-ordered emission.  DVE: A per group as x lands, B
        # behind the sigmoid chunks; C+fix follow on DVE or Pool; stores
        # ride the SWDGE ring behind the loads.
        def finish(g):
            # C/fix/store for both halves on the owning engine
            eng = nc.gpsimd if g in pool_c else nc.vector
            for h in range(2):
                c_op(eng, g, h)
                fix_op(eng, g, h)
                store(g, h)

        for i, g in enumerate(gorder):
            if g in swcs:
                swc_copy(g, 0)
                swc_copy(g, 1)
            a_op(g, 0)
            a_op(g, 1)
            if i >= 1:
                gp = gorder[i - 1]
                b_op(gp, 0)
                b_op(gp, 1)
                finish(gp)
        gl = gorder[-1]
        b_op(gl, 0)
        b_op(gl, 1)
        finish(gl)

    nc.finalize()
    return nc


_GRAPH_CACHE = {}


def _install_ntff_hook_shim():
    """The agent image's ``antenv`` lacks ``axon_hooks``; recreate it so
    run_bass_kernel_spmd(trace=True) can capture NTFF profiles."""
    import sys
    import types

    if "antenv.axon_hooks" in sys.modules:
        return
    try:
        import antenv
        from trn_agent_boot.trn_boot import _ntff_profile_via_ctypes
    except ImportError:
        return
    hook = _ntff_profile_via_ctypes("/opt/axon/libaxon_pjrt.so")
    mod = types.ModuleType("antenv.axon_hooks")
    mod._hook = hook

    def set_axon_ntff_profile_hook(h):
        mod._hook = h

    def get_axon_ntff_profile_hook():
        return mod._hook

    mod.set_axon_ntff_profile_hook = set_axon_ntff_profile_hook
    mod.get_axon_ntff_profile_hook = get_axon_ntff_profile_hook
    sys.modules["antenv.axon_hooks"] = mod
    antenv.axon_hooks = mod


def _constants(w1, b1, w2, b2):
    import ml_dtypes

    # w1t2 [18, 32]: rows 0/1 = center tap (em_b0/em_b1), rows 2-9/10-17
    # = taps [0,1,2,3,5,6,7,8] of b0/b1; cols 0-15 = b0 outputs, 16-31 =
    # b1.  The 1/C edge-mean scale is folded in (p1w2 holds channel sums).
    w1tc = (w1.T / C).astype(np.float32)  # [9, 16]
    perm8 = [0, 1, 2, 3, 5, 6, 7, 8]
    w1t2 = np.zeros((18, 32), np.float32)
    w1t2[0, 0:16] = w1tc[4]
    w1t2[1, 16:32] = w1tc[4]
    for k, tap in enumerate(perm8):
        w1t2[2 + k, 0:16] = w1tc[tap]
        w1t2[10 + k, 16:32] = w1tc[tap]

    # dw2p2 [3][96, 128]: row 3c+dj = (w2[0]-w2[1])[c, 3d+dj] replicated
    # to 64 cols (b0 block), rows 48+ same for b1 in cols 64-127.
    dw2 = (w2[0] - w2[1]).reshape(16, 3, 3)
    dw2p2 = np.zeros((3, 96, 128), np.float32)
    for d in range(3):
        base = dw2[:, d, :].reshape(48, 1)
        dw2p2[d, 0:48, 0:64] = base
        dw2p2[d, 48:96, 64:128] = base

    ones2 = np.zeros((128, 2), np.float32)
    ones2[0:64, 0] = 1.0
    ones2[64:128, 1] = 1.0

    return {
        "w1t2": w1t2.astype(ml_dtypes.bfloat16),
        "dw2p2": dw2p2.reshape(288, 128).astype(ml_dtypes.bfloat16),
        "ones2": ones2.astype(ml_dtypes.bfloat16),
        "b1x2": np.tile(b1, 2).reshape(32, 1).astype(np.float32),
        "db2": np.full((128, 1), b2[0] - b2[1], dtype=np.float32),
    }


def kernel(**inputs):
    from concourse.bass_utils import run_bass_kernel_spmd

    _install_ntff_hook_shim()

    x = np.ascontiguousarray(inputs["x"], dtype=np.float32).reshape(B, C, HW)
    eg = np.ascontiguousarray(inputs["edge_guidance"], dtype=np.float32).reshape(
        B, C, HW
    )
    w1 = np.asarray(inputs["w1"], dtype=np.float32).reshape(16, 9)
    b1 = np.asarray(inputs["b1"], dtype=np.float32).reshape(16)
    w2 = np.asarray(inputs["w2"], dtype=np.float32).reshape(2, 16, 9)
    b2 = np.asarray(inputs["b2"], dtype=np.float32).reshape(2)
    shifts_h = np.asarray(inputs["shifts_h"]).astype(np.int64)
    shifts_w = np.asarray(inputs["shifts_w"]).astype(np.int64)

    consts = _constants(w1, b1, w2, b2)

    key = (tuple(shifts_h.tolist()), tuple(shifts_w.tolist()))
    if key not in _GRAPH_CACHE:
        _GRAPH_CACHE[key] = _build(shifts_h, shifts_w)
    nc = _GRAPH_CACHE[key]

    in_maps = []
    for i in range(NCORES):
        bsl = slice(i * BLOC, (i + 1) * BLOC)
        in_maps.append(
            {
                "x": np.ascontiguousarray(x[bsl]),
                "eg": np.ascontiguousarray(eg[bsl]),
                **consts,
            }
        )

    import os

    trace = os.environ.get("BASS_KERNEL_TRACE", "0") == "1"
    try:
        res = run_bass_kernel_spmd(nc, in_maps, list(range(NCORES)), trace=trace)
    except Exception:
        if not trace:
            raise
        res = run_bass_kernel_spmd(nc, in_maps, list(range(NCORES)), trace=False)

    LAST_RESULT["exec_time_ns"] = getattr(res, "exec_time_ns", None)
    LAST_RESULT["profile_json"] = getattr(res, "profile_json", None)

    out = np.concatenate(
        [np.asarray(res.results[i]["out"]) for i in range(NCORES)], axis=0
    )
    return out.astype(np.float32).reshape(B, C, H, W)


# revision 13
# speedup vs baseline: 1.4558x; 1.0999x over previous
"""Trainium2 Bass kernel for nn_AdaptiveDirectionShift (v4).

Reference computation (B=16, C=320, H=W=64, G=5 groups of 64 channels):
  xn = zero-pad x spatially by 2          -> [B,C,68,68]
  em = mean_c(edge_guidance)              -> [B,1,64,64]
  h  = relu(conv3x3(em, w1, b1))          -> [B,16,64,64]
  dl = conv3x3(h, w2, b2)                 -> [B,2,64,64]
  dw = softmax(dl, axis=1)                -> wH = sigmoid(dl0-dl1), wW = 1-wH
  sh = roll rows of xn per group by shifts_h, crop -> [B,C,64,64]
  sw = roll cols of xn per group by shifts_w, crop
  out = wH*sh + wW*sw = sw + wH*(sh-sw)

Strategy: data-parallel over batch, 2 batches per core, no collectives.

v4 design (baseline v2: 123.8us; v3: 195.8us — HWDGE DRAM stores turned
out to cost ~10us of ucode descriptor generation each):
 * ALL HBM traffic (casting loads + stores) on the gpsimd SWDGE ring,
   whose Q7 CounterMachine generates the 128-row DRAM descriptors fast.
   HWDGE rings carry only SBUF->SBUF traffic (taps, conv patches,
   aligned shifted copies) where their descriptor RTL is fast.
 * batch-fused gate via block-diagonal weights: em [128->2] with a
   [128,2] ones mask, conv1 [18->32], conv2 [96->128]; one sigmoid per
   512-chunk emits both batches' wh.
 * em accumulates over the 5 channel groups ON THE PE: 8 PSUM chunk
   tiles stay open while per-group matmuls accumulate as each eg tile
   lands.  Costs zero DVE time.
 * the gate is pipelined in image-row halves: eg loads arrive in
   halves, and em/taps/conv1/p2c/conv2/sigmoid chase so the first
   sigmoid chunks are ready ~25us before the last eg byte's full-image
   gate would be.
 * x tiles carry 2 full guard rows per side (GB=128), so every row
   shift |s|<=2 is one flat offset read; groups with ODD col-shift get
   an aligned SBUF->SBUF copy so all DVE combine ops run in 2x bf16
   mode.  Only tiny col-edge fixups remain.
 * the Pool engine takes over C (+fixups) for two late-arriving groups
   once its load-dispatch work is done, shortening the DVE tail; the
   last-loaded group is the one needing no aligned copy.

Shift values are read host-side and baked into the access patterns
(compile-time specialization, like shapes).
"""

import numpy as np

B, C, H, W = 16, 320, 64, 64
HW = H * W
NCORES = 8
BLOC = B // NCORES  # 2 batches per core
G, CG = 5, 64       # channel groups
PAD = 2

PL = 66             # padded line width for conv tensors
FLAT = 64 * PL      # 4224: flat length of 64 rows of 66-wide lines
GB = 2 * W          # 128: two full guard rows each side of the x tiles
GTOT = GB + HW + GB  # 4352
P2L = PL * PL - 2   # 4354

# combine halves (A/B/C/store ranges): half0 must only need x rows 0-31
HALF0 = 30 * W      # 1920
HALVES = [(0, HALF0), (HALF0, HW)]
XH = HW // 2        # loads come in 2048-elem (32-row) halves

# gate pipeline half boundaries (see derivation in comments below)
TAPS_CUT = 30 * PL   # taps half0 reads em rows <= 31
P2C_CUT = 25 * PL - 2  # p2c half0 reads h_pad2 rows <= 24 (relu chunks 0-2)
CONV1_H0 = 3         # conv1 chunks 0-2 need taps half0 only
CONV2_H0 = 2         # conv2 chunks 0-1 need p2c half0 only

LAST_RESULT = {}


def _build(shifts_h, shifts_w):
    from contextlib import ExitStack

    import concourse.bass as bass
    import concourse.tile as tile
    from concourse import bacc, mybir

    f32 = mybir.dt.float32
    bf16 = mybir.dt.bfloat16
    nc = bacc.Bacc(None, target_bir_lowering=False)

    x_ext = nc.declare_dram_parameter("x", [BLOC, C, HW], f32, isOutput=False)
    eg_ext = nc.declare_dram_parameter("eg", [BLOC, C, HW], f32, isOutput=False)
    w1t2_ext = nc.declare_dram_parameter("w1t2", [18, 32], bf16, isOutput=False)
    dw2p2_ext = nc.declare_dram_parameter("dw2p2", [3 * 96, 128], bf16, isOutput=False)
    ones2_ext = nc.declare_dram_parameter("ones2", [128, 2], bf16, isOutput=False)
    b1x2_ext = nc.declare_dram_parameter("b1x2", [32, 1], f32, isOutput=False)
    db2_ext = nc.declare_dram_parameter("db2", [128, 1], f32, isOutput=False)
    out_ext = nc.declare_dram_parameter("out", [BLOC, C, HW], bf16, isOutput=True)

    sh_l = [int(v) for v in shifts_h]
    sw_l = [int(v) for v in shifts_w]
    assert all(-PAD <= v <= PAD for v in sh_l + sw_l)

    # load/combine order: put a group needing no aligned copy (even t)
    # last if one exists — the last-arriving group has the shortest
    # post-arrival chain.  The 3rd/4th-loaded groups may run their C as
    # SBUF->SBUF accumulate DMAs to shorten the DVE tail.
    evens = [g for g in range(G) if sw_l[g] % 2 == 0]
    odds = [g for g in range(G) if sw_l[g] % 2 != 0]
    gorder = odds + evens if evens else list(range(G))
    import os

    if os.environ.get("V5_C_DMA", "1") == "1":
        c_dma = set(gorder[2:4])
    else:
        c_dma = set()

    def raw_ap(tile_ap, part0, nparts, offset, free_dims):
        pstep = tile_ap.ap[0][0]
        return bass.AP(
            tensor=tile_ap.tensor,
            offset=tile_ap.offset + pstep * part0 + offset,
            ap=[[pstep, nparts]] + [list(d) for d in free_dims],
        )

    with tile.TileContext(nc) as tc, ExitStack() as ctx:
        singles = ctx.enter_context(tc.tile_pool(name="singles", bufs=1))
        gate_pool = ctx.enter_context(tc.tile_pool(name="gatep", bufs=1))
        # [128, HW] bf16 buffers: 5 eg group tiles, then the 5 e tiles
        # reuse them (lifetimes don't overlap)
        big = ctx.enter_context(tc.tile_pool(name="big", bufs=5))
        # single PSUM pool: 8 em chunk tiles live at once, then conv1/
        # conv2 tiles cycle through the freed banks
        ps = ctx.enter_context(tc.tile_pool(name="ps", bufs=8, space="PSUM"))

        # ---- constants (tiny, on the sync HWDGE ring) ----
        ones2 = singles.tile([128, 2], bf16, tag="ones2")
        nc.sync.dma_start(out=ones2, in_=ones2_ext[:, :])
        w1t2 = singles.tile([18, 32], bf16, tag="w1t2")
        nc.sync.dma_start(out=w1t2, in_=w1t2_ext[:, :])
        dw2p2 = []
        for d in range(3):
            dwt = singles.tile([96, 128], bf16, tag=f"dw2p2{d}", name=f"dw2p2{d}")
            nc.sync.dma_start(out=dwt, in_=dw2p2_ext[96 * d : 96 * d + 96, :])
            dw2p2.append(dwt)
        b1x2 = singles.tile([32, 1], f32, tag="b1x2")
        nc.sync.dma_start(out=b1x2, in_=b1x2_ext[:, :])
        db2 = singles.tile([128, 1], f32, tag="db2")
        nc.sync.dma_start(out=db2, in_=db2_ext[:, :])

        # ---- persistent gate tensors ----
        # p1w2: partitions 0/1 = em images of b0/b1 (64 rows x 66-wide
        # lines), partitions 2-9 = b0's 8 other conv taps, 10-17 = b1's.
        p1w2 = gate_pool.tile([18, FLAT], bf16, tag="p1w2")
        h_pad2 = gate_pool.tile([32, PL, PL], bf16, tag="h_pad2")
        blk = gate_pool.tile([96, P2L], bf16, tag="p2c")
        whb2 = gate_pool.tile([128, HW], bf16, tag="whb2")

        # zero only the strips the writers below do not cover
        nc.vector.memset(raw_ap(p1w2, 0, 18, 0, [[1, 67]]), 0.0)
        nc.vector.memset(raw_ap(p1w2, 0, 18, FLAT - 67, [[1, 67]]), 0.0)
        nc.vector.memset(raw_ap(p1w2, 0, 2, 0, [[PL, 64], [1, 1]]), 0.0)
        nc.vector.memset(raw_ap(p1w2, 0, 2, PL - 1, [[PL, 64], [1, 1]]), 0.0)
        nc.vector.memset(raw_ap(h_pad2, 0, 32, 0, [[1, PL]]), 0.0)
        nc.vector.memset(raw_ap(h_pad2, 0, 32, 65 * PL, [[1, PL]]), 0.0)
        nc.vector.memset(raw_ap(h_pad2, 0, 32, PL, [[PL, 64], [1, 1]]), 0.0)
        nc.vector.memset(raw_ap(h_pad2, 0, 32, PL + 65, [[PL, 64], [1, 1]]), 0.0)

        # dummy activation pulls the sigmoid ACT_TABLE load off the path
        warm = gate_pool.tile([1, 1], f32, tag="warm")
        nc.vector.memset(warm, 0.0)
        nc.scalar.activation(
            out=warm, in_=warm, func=mybir.ActivationFunctionType.Sigmoid
        )

        # ---- x tiles + guard zeroing (DVE, cheap) ----
        xgs = {}
        for g in range(G):
            xg = singles.tile([128, GTOT], bf16, tag=f"xg{g}", name=f"xg{g}")
            nc.vector.memset(raw_ap(xg, 0, 128, 0, [[1, GB]]), 0.0)
            nc.vector.memset(raw_ap(xg, 0, 128, GB + HW, [[1, GB]]), 0.0)
            xgs[g] = xg

        # ---- SWDGE ring: eg halves, then x (gorder), stores come later
        egts = []
        for g in range(G):
            egts.append(big.tile([128, HW], bf16, tag="bigbuf", name=f"egt{g}"))
        for h in range(2):
            n0 = h * XH
            for g in range(G):
                for b in range(BLOC):
                    nc.gpsimd.dma_start(
                        out=raw_ap(egts[g], b * CG, CG, n0, [[1, XH]]),
                        in_=eg_ext[b, g * CG : (g + 1) * CG, n0 : n0 + XH],
                    )
        for g in gorder:
            for h in range(2):
                n0 = h * XH
                for b in range(BLOC):
                    nc.gpsimd.dma_start(
                        out=raw_ap(xgs[g], b * CG, CG, GB + n0, [[1, XH]]),
                        in_=x_ext[b, g * CG : (g + 1) * CG, n0 : n0 + XH],
                    )

        # ================= gate network (batch-fused, half-pipelined) ==
        NJ = HW // 512  # 8 chunks of 512 px (8 image rows each)
        em_ps = [None] * NJ

        def em_stage(jlo, jhi):
            # accumulate the channel sums on the PE: per group, chunk
            # matmuls [128 -> 2] land in open PSUM tiles as eg arrives
            for j in range(jlo, jhi):
                em_ps[j] = ps.tile([2, 512], f32, tag="ps", name=f"em{j}")
            for g in range(G):
                for j in range(jlo, jhi):
                    nc.tensor.matmul(
                        em_ps[j], ones2, egts[g][:, j * 512 : (j + 1) * 512],
                        start=(g == 0), stop=(g == G - 1),
                    )
            for j in range(jlo, jhi):
                r0 = (j * 512) // W
                dst = raw_ap(p1w2, 0, 2, r0 * PL + 1, [[PL, 8], [1, 64]])
                nc.scalar.copy(
                    out=dst,
                    in_=em_ps[j][0:2, :].rearrange("p (r c) -> p r c", c=64),
                )

        TAPGROUPS = (
            (nc.sync, 0, 3, -PL - 1, 1),   # taps 0,1,2: delta -67,-66,-65
            (nc.scalar, 3, 2, -1, 2),      # taps 3,5:   delta -1,+1
            (nc.sync, 5, 3, PL - 1, 1),    # taps 6,7,8: delta 65,66,67
        )

        def taps_stage(half):
            for b in range(BLOC):
                for ring, tapoff, np_, base, estep in TAPGROUPS:
                    lo = max(0, -base)
                    hi = FLAT - max(0, base + estep * (np_ - 1))
                    a0, a1 = (lo, TAPS_CUT) if half == 0 else (TAPS_CUT, hi)
                    src = raw_ap(
                        p1w2, b, 1, a0 + base, [[estep, np_], [1, a1 - a0]]
                    )
                    dst = raw_ap(p1w2, 2 + 8 * b + tapoff, np_, a0, [[1, a1 - a0]])
                    ring.dma_start(out=dst, in_=src)

        def conv1_stage(jlo, jhi):
            for j in range(jlo, jhi):
                h_ps = ps.tile([32, 512], f32, tag="ps", name=f"h{j}")
                r0 = (j * 512) // W
                rhs = raw_ap(p1w2, 0, 18, r0 * PL + 1, [[PL, 8], [1, 64]])
                nc.tensor.matmul(h_ps, w1t2, rhs, start=True, stop=True)
                nc.scalar.activation(
                    out=h_pad2[0:32, 1 + r0 : 9 + r0, 1:65],
                    in_=h_ps[0:32, :].rearrange("p (r c) -> p r c", c=64),
                    func=mybir.ActivationFunctionType.Relu,
                    bias=b1x2[0:32, 0:1],
                )

        def p2c_stage(half):
            # h1 rides the scalar ring only: a sync-ring p2c-h1 would
            # head-of-line block the swc copies behind it (it waits on
            # the last relu)
            rings = ((0, nc.sync), (1, nc.scalar)) if half == 0 else (
                (0, nc.scalar), (1, nc.scalar))
            for b, ring in rings:
                a0, a1 = (0, P2C_CUT) if half == 0 else (P2C_CUT, P2L)
                ring.dma_start(
                    out=raw_ap(blk, 48 * b, 48, a0, [[1, a1 - a0]]),
                    in_=raw_ap(h_pad2, 16 * b, 16, a0, [[1, 3], [1, a1 - a0]]),
                )

        def conv2_stage(jlo, jhi):
            for j in range(jlo, jhi):
                r0 = (j * 512) // W
                d_ps = ps.tile([128, 512], f32, tag="ps", name=f"d{j}")
                for d in range(3):
                    rhs = raw_ap(blk, 0, 96, (r0 + d) * PL, [[PL, 8], [1, 64]])
                    nc.tensor.matmul(
                        d_ps, dw2p2[d], rhs, start=(d == 0), stop=(d == 2)
                    )
                nc.scalar.activation(
                    out=raw_ap(whb2, 0, 128, j * 512, [[1, 512]]),
                    in_=d_ps[0:128, :],
                    func=mybir.ActivationFunctionType.Sigmoid,
                    bias=db2[0:128, 0:1],
                )

        # half-pipelined gate.  Emission order is engine-FIFO order:
        #  - PE: em h1 right after conv1 h0 (NOT behind conv2 h0, whose
        #    patch DMA data crawls behind the load stream)
        #  - ACT: em-h1 drains + taps h1 before the first sigmoids, so
        #    the h1 tap DMAs start as early as possible
        em_stage(0, NJ // 2)
        taps_stage(0)
        conv1_stage(0, CONV1_H0)
        p2c_stage(0)
        em_stage(NJ // 2, NJ)
        taps_stage(1)
        conv2_stage(0, CONV2_H0)
        conv1_stage(CONV1_H0, NJ)
        p2c_stage(1)
        conv2_stage(CONV2_H0, NJ)

        # ================= shifted combines (per group) ================
        # Per group g: sh = flat read at GB - W*s, sw = flat read at
        # GB - t (aligned copy swc for odd t, with its col-edge runs
        # zeroed so sw==0 holds EXACTLY where the reference pads: the
        # A/B/C chain then needs no fixup at all).
        #   A: e = sh - sw       B: e *= wh       C: e += sw
        # A/B on the DVE; C on the DVE or as an SBUF->SBUF accumulate
        # DMA (c_dma groups); only even-t groups keep a col-edge fixup.
        e_ts = {}
        for g in gorder:
            e_ts[g] = big.tile([128, HW], bf16, tag="bigbuf", name=f"e{g}")
        swcs = {}
        for g in odds:
            swcs[g] = singles.tile([128, HW], bf16, tag=f"swc{g}", name=f"swc{g}")

        def swc_copy(g, h):
            lo, hi = HALVES[h]
            nc.sync.dma_start(
                out=raw_ap(swcs[g], 0, 128, lo, [[1, hi - lo]]),
                in_=raw_ap(xgs[g], 0, 128, GB - sw_l[g] + lo, [[1, hi - lo]]),
            )

        def swc_zero_edges(g, h):
            # zero the cols where the reference's col-roll pads with 0
            t = sw_l[g]
            rlo, rhi = (0, 30) if h == 0 else (30, 64)
            j0, nj = (0, t) if t > 0 else (64 + t, -t)
            nc.vector.memset(
                raw_ap(swcs[g], 0, 128, rlo * W + j0, [[W, rhi - rlo], [1, nj]]),
                0.0,
            )

        def sw_ap(g, lo, hi):
            if g in swcs:
                return raw_ap(swcs[g], 0, 128, lo, [[1, hi - lo]])
            return raw_ap(xgs[g], 0, 128, GB - sw_l[g] + lo, [[1, hi - lo]])

        def a_op(g, h):
            lo, hi = HALVES[h]
            nc.vector.tensor_sub(
                raw_ap(e_ts[g], 0, 128, lo, [[1, hi - lo]]),
                raw_ap(xgs[g], 0, 128, GB - W * sh_l[g] + lo, [[1, hi - lo]]),
                sw_ap(g, lo, hi),
            )

        # B runs in three sigmoid-chasing pieces (chunk granularity of
        # whb2 is 512; pieces align to 1024/1024/2048)
        BPIECES = [(0, 1024), (1024, 2048), (2048, HW)]

        def b_op(g, p):
            lo, hi = BPIECES[p]
            nc.vector.tensor_mul(
                raw_ap(e_ts[g], 0, 128, lo, [[1, hi - lo]]),
                raw_ap(e_ts[g], 0, 128, lo, [[1, hi - lo]]),
                raw_ap(whb2, 0, 128, lo, [[1, hi - lo]]),
            )

        CHALVES = [(0, 2048), (2048, HW)]

        def c_op(g, h):
            lo, hi = CHALVES[h]
            if g in c_dma:
                nc.gpsimd.dma_start(
                    out=raw_ap(e_ts[g], 0, 128, lo, [[1, hi - lo]]),
                    in_=sw_ap(g, lo, hi),
                    accum_op=mybir.AluOpType.add,
                )
            else:
                nc.vector.tensor_add(
                    raw_ap(e_ts[g], 0, 128, lo, [[1, hi - lo]]),
                    raw_ap(e_ts[g], 0, 128, lo, [[1, hi - lo]]),
                    sw_ap(g, lo, hi),
                )

        def fix_op(g, h):
            # even-t groups read sw straight from xg: overwrite the
            # wrapped col-edge with the true value wh*sh
            t = sw_l[g]
            if t == 0 or g in swcs:
                return
            rlo, rhi = (0, 32) if h == 0 else (32, 64)
            nr = rhi - rlo
            j0, nj = (0, t) if t > 0 else (64 + t, -t)
            f0 = rlo * W + j0
            er = raw_ap(e_ts[g], 0, 128, f0, [[W, nr], [1, nj]])
            whr = raw_ap(whb2, 0, 128, f0, [[W, nr], [1, nj]])
            shr = raw_ap(
                xgs[g], 0, 128, GB - W * sh_l[g] + f0, [[W, nr], [1, nj]]
            )
            nc.vector.tensor_mul(er, whr, shr)

        def store(g, h):
            lo, hi = CHALVES[h]
            nc.gpsimd.dma_start(
                out=out_ext[:, g * CG : (g + 1) * CG, lo:hi],
                in_=raw_ap(e_ts[g], 0, 128, lo, [[1, hi - lo]]),
            )

        # Readiness-ordered emission on the in-order engines:
        #  - sync ring: swc copies as each x group lands
        #  - DVE: A as x+swc land, B pieces behind the sigmoid chunks,
        #    then C (non-dma groups) and even-t fixups
        #  - gpsimd ring (behind the loads): accumulate-C for c_dma
        #    groups and all stores, in combine order
        def emit_a(g):
            if g in swcs:
                swc_copy(g, 0)
                swc_zero_edges(g, 0)
                swc_copy(g, 1)
                swc_zero_edges(g, 1)
            a_op(g, 0)
            a_op(g, 1)

        def emit_bcs(g):
            for p in range(3):
                b_op(g, p)
            for h in range(2):
                c_op(g, h)
                fix_op(g, h)
                store(g, h)

        emit_a(gorder[0])
        emit_a(gorder[1])
        emit_bcs(gorder[0])
        emit_a(gorder[2])
        emit_bcs(gorder[1])
        emit_a(gorder[3])
        emit_bcs(gorder[2])
        emit_a(gorder[4])
        emit_bcs(gorder[3])
        emit_bcs(gorder[4])

    nc.finalize()
    return nc


_GRAPH_CACHE = {}


def _install_ntff_hook_shim():
    """The agent image's ``antenv`` lacks ``axon_hooks``; recreate it so
    run_bass_kernel_spmd(trace=True) can capture NTFF profiles."""
    import sys
    import types

    if "antenv.axon_hooks" in sys.modules:
        return
    try:
        import antenv
        from trn_agent_boot.trn_boot import _ntff_profile_via_ctypes
    except ImportError:
        return
    hook = _ntff_profile_via_ctypes("/opt/axon/libaxon_pjrt.so")
    mod = types.ModuleType("antenv.axon_hooks")
    mod._hook = hook

    def set_axon_ntff_profile_hook(h):
        mod._hook = h

    def get_axon_ntff_profile_hook():
        return mod._hook

    mod.set_axon_ntff_profile_hook = set_axon_ntff_profile_hook
    mod.get_axon_ntff_profile_hook = get_axon_ntff_profile_hook
    sys.modules["antenv.axon_hooks"] = mod
    antenv.axon_hooks = mod


def _constants(w1, b1, w2, b2):
    import ml_dtypes

    # w1t2 [18, 32]: rows 0/1 = center tap (em_b0/em_b1), rows 2-9/10-17
    # = taps [0,1,2,3,5,6,7,8] of b0/b1; cols 0-15 = b0 outputs, 16-31 =
    # b1.  The 1/C edge-mean scale is folded in (p1w2 holds channel sums).
    w1tc = (w1.T / C).astype(np.float32)  # [9, 16]
    perm8 = [0, 1, 2, 3, 5, 6, 7, 8]
    w1t2 = np.zeros((18, 32), np.float32)
    w1t2[0, 0:16] = w1tc[4]
    w1t2[1, 16:32] = w1tc[4]
    for k, tap in enumerate(perm8):
        w1t2[2 + k, 0:16] = w1tc[tap]
        w1t2[10 + k, 16:32] = w1tc[tap]

    # dw2p2 [3][96, 128]: row 3c+dj = (w2[0]-w2[1])[c, 3d+dj] replicated
    # to 64 cols (b0 block), rows 48+ same for b1 in cols 64-127.
    dw2 = (w2[0] - w2[1]).reshape(16, 3, 3)
    dw2p2 = np.zeros((3, 96, 128), np.float32)
    for d in range(3):
        base = dw2[:, d, :].reshape(48, 1)
        dw2p2[d, 0:48, 0:64] = base
        dw2p2[d, 48:96, 64:128] = base

    ones2 = np.zeros((128, 2), np.float32)
    ones2[0:64, 0] = 1.0
    ones2[64:128, 1] = 1.0

    return {
        "w1t2": w1t2.astype(ml_dtypes.bfloat16),
        "dw2p2": dw2p2.reshape(288, 128).astype(ml_dtypes.bfloat16),
        "ones2": ones2.astype(ml_dtypes.bfloat16),
        "b1x2": np.tile(b1, 2).reshape(32, 1).astype(np.float32),
        "db2": np.full((128, 1), b2[0] - b2[1], dtype=np.float32),
    }


def kernel(**inputs):
    from concourse.bass_utils import run_bass_kernel_spmd

    _install_ntff_hook_shim()

    x = np.ascontiguousarray(inputs["x"], dtype=np.float32).reshape(B, C, HW)
    eg = np.ascontiguousarray(inputs["edge_guidance"], dtype=np.float32).reshape(
        B, C, HW
    )
    w1 = np.asarray(inputs["w1"], dtype=np.float32).reshape(16, 9)
    b1 = np.asarray(inputs["b1"], dtype=np.float32).reshape(16)
    w2 = np.asarray(inputs["w2"], dtype=np.float32).reshape(2, 16, 9)
    b2 = np.asarray(inputs["b2"], dtype=np.float32).reshape(2)
    shifts_h = np.asarray(inputs["shifts_h"]).astype(np.int64)
    shifts_w = np.asarray(inputs["shifts_w"]).astype(np.int64)

    consts = _constants(w1, b1, w2, b2)

    key = (tuple(shifts_h.tolist()), tuple(shifts_w.tolist()))
    if key not in _GRAPH_CACHE:
        _GRAPH_CACHE[key] = _build(shifts_h, shifts_w)
    nc = _GRAPH_CACHE[key]

    in_maps = []
    for i in range(NCORES):
        bsl = slice(i * BLOC, (i + 1) * BLOC)
        in_maps.append(
            {
                "x": np.ascontiguousarray(x[bsl]),
                "eg": np.ascontiguousarray(eg[bsl]),
                **consts,
            }
        )

    import os

    trace = os.environ.get("BASS_KERNEL_TRACE", "0") == "1"
    try:
        res = run_bass_kernel_spmd(nc, in_maps, list(range(NCORES)), trace=trace)
    except Exception:
        if not trace:
            raise
        res = run_bass_kernel_spmd(nc, in_maps, list(range(NCORES)), trace=False)

    LAST_RESULT["exec_time_ns"] = getattr(res, "exec_time_ns", None)
    LAST_RESULT["profile_json"] = getattr(res, "profile_json", None)

    out = np.concatenate(
        [np.asarray(res.results[i]["out"]) for i in range(NCORES)], axis=0
    )
    return out.astype(np.float32).reshape(B, C, H, W)
